# revision 14
# baseline (speedup 1.0000x reference)
"""DoRA linear layer on 8 TRN2 NeuronCores.

out = (magnitude / ||W + s*B@A||_row) * (x @ (W + s*B@A)^T),  s = alpha/rank = 2.

Identity used: the reference's
    dora_out + base_out = mag_norm_scale * (base_out + s * lora_out)
                        = scale_o * (x @ W_adapted^T)

Sharding: TENSOR-PARALLEL on out_dim (per the sharding hint): core k owns
output columns [512k, 512(k+1)), x is replicated (streamed), W/lora_b/
magnitude are column-sharded.  This makes the norm/scale computation fully
LOCAL to each core — no cross-core exchange of any kind.

On-device, each core materializes its adapted weight column ONCE:
    W_ad^T = W^T + A^T @ B2^T      (32 K=16 matmuls + 32 DVE adds, fp16)
after which
  * the main GEMM is 64 token-tiles x ONE PSUM chain of 32 fp16 matmuls
    (no separate rank-16 path, no x@A^T precompute), and
  * the row norm is simply rowsum(W_ad^2): 32 DVE squares + 32 ones-matmul
    accumulations into a [1,512] PSUM, consistent to the bit with the
    weights the GEMM consumes.
scale = mag / sqrt(nsq) broadcast once into a [128,512] tile; every PSUM
drain is a single fused tensor_mul.

Host side only reshapes/transposes (layout prep), casts fp32 -> fp16
(accuracy budget is rel_err < 2e-2; fp16 gives ~4e-4), and concatenates
the per-core output column blocks.
"""

import sys

sys.path.insert(0, "/opt/trn_rl_repo")

import numpy as np

import concourse.bass as bass  # noqa: F401  (import keeps bass registered)
from concourse import bacc
import concourse.mybir as mybir
from concourse.tile import TileContext
from concourse.bass_utils import run_bass_kernel_spmd

FP32 = mybir.dt.float32
FP16 = mybir.dt.float16

NCORES = 8
TOK = 8192          # 4 * 2048 tokens total, all processed by every core
DIN = 4096
DOUT = 4096
RANK = 16
SCALING = 32.0 / 16

NI = DIN // 128     # 32 contraction blocks
OC = DOUT // NCORES  # 512 output columns per core
NB = TOK // 128     # 64 token tiles per core


def _build_program():
    nc = bacc.Bacc("TRN2", target_bir_lowering=False, debug=False,
                   num_devices=NCORES)

    # x in token-block-major layout: block t -> [128 part, NI*128] contiguous
    xb_d = nc.dram_tensor("xb", [NB, 128, NI * 128], FP16,
                          kind="ExternalInput")
    wt_d = nc.dram_tensor("wt", [NI, 128, OC], FP16, kind="ExternalInput")
    atr_d = nc.dram_tensor("atr", [RANK, NI, 128], FP16, kind="ExternalInput")
    b2n_d = nc.dram_tensor("b2n", [RANK, OC], FP16, kind="ExternalInput")
    magn_d = nc.dram_tensor("magn", [1, OC], FP32, kind="ExternalInput")
    out_d = nc.dram_tensor("out", [TOK, OC], FP32, kind="ExternalOutput")
    srow_d = nc.dram_tensor("srow_scratch", [1, OC], FP32)

    with TileContext(nc) as tc:
        with (
            tc.tile_pool(name="const", bufs=1) as const,
            tc.tile_pool(name="xbp", bufs=8) as xbp,
            tc.tile_pool(name="wp", bufs=6) as wp,
            tc.tile_pool(name="wadp", bufs=32) as wadp,
            tc.tile_pool(name="wsqp", bufs=4) as wsqp,
            tc.tile_pool(name="outp", bufs=10) as outp,
            tc.tile_pool(name="mp", bufs=5, space="PSUM") as mp,
            tc.tile_pool(name="plp", bufs=2, space="PSUM") as plp,
            tc.tile_pool(name="sp", bufs=1, space="PSUM") as sp,
        ):
            atr = const.tile([RANK, NI, 128], FP16)
            nc.sync.dma_start(atr[:], atr_d[:])
            b2n_sb = const.tile([RANK, OC], FP16)
            nc.sync.dma_start(b2n_sb[:], b2n_d[:])
            magn_sb = const.tile([1, OC], FP32)
            nc.sync.dma_start(magn_sb[:], magn_d[:])
            ones128 = const.tile([128, 1], FP16)
            nc.vector.memset(ones128[:], 1.0)

            # --- W_ad^T = W^T + A^T @ B2^T, per 128-row i-block ----------
            # also accumulate nsq = colsum(W_ad^2) as the blocks appear,
            # and interleave the first NPRO main chains so the PE starts
            # streaming the GEMM as soon as each W_ad tile exists
            NPRO = 3
            pro_ps = []
            pro_xb = []
            for t in range(NPRO):
                xb = xbp.tile([128, NI * 128], FP16, tag="xb", name=f"xb{t}")
                nc.sync.dma_start(xb[:], xb_d[t])
                pro_xb.append(xb)
                pro_ps.append(mp.tile([128, OC], FP32, tag="mp",
                                      name=f"pm{t}"))
            wads = []
            ps_nsq = sp.tile([1, OC], FP32, tag="nsq", name="psnsq")
            for ib in range(NI):
                w_t = wp.tile([128, OC], FP16, tag="w", name=f"w{ib}")
                nc.sync.dma_start(w_t[:], wt_d[ib])
                ps_l = plp.tile([128, OC], FP32, tag="pl", name=f"pl{ib}")
                nc.tensor.matmul(ps_l[:], atr[:, ib, :], b2n_sb[:],
                                 start=True, stop=True)
                wad = wadp.tile([128, OC], FP16, tag="wad", name=f"wad{ib}")
                nc.vector.tensor_add(wad[:], ps_l[:], w_t[:])
                wads.append(wad)
                wsq = wsqp.tile([128, OC], FP16, tag="wsq", name=f"wsq{ib}")
                nc.vector.tensor_mul(wsq[:], wad[:], wad[:])
                nc.tensor.matmul(ps_nsq[:], ones128[:], wsq[:],
                                 start=(ib == 0), stop=(ib == NI - 1))
                for t in range(NPRO):
                    nc.tensor.matmul(
                        pro_ps[t][:],
                        pro_xb[t][:, ib * 128:(ib + 1) * 128],
                        wad[:], start=(ib == 0), stop=(ib == NI - 1))

            # --- scale = mag / sqrt(nsq), broadcast over partitions ------
            nrmrow = const.tile([1, OC], FP32)
            srow = const.tile([1, OC], FP32)
            nc.scalar.activation(nrmrow[:], ps_nsq[:],
                                 mybir.ActivationFunctionType.Sqrt)
            nc.vector.reciprocal(nrmrow[:], nrmrow[:])
            nc.vector.tensor_mul(srow[:], nrmrow[:], magn_sb[:])
            sbc = const.tile([128, OC], FP32)
            nc.gpsimd.dma_start(srow_d[:], srow[:])
            _sl = srow_d[:]
            srow_bcast = bass.AP(
                tensor=_sl.tensor, offset=_sl.offset,
                ap=[[0, 128], [1, OC]])
            nc.gpsimd.dma_start(sbc[:], srow_bcast)

            # prologue chains: scale-multiply + store
            for t in range(NPRO):
                o_t = outp.tile([128, OC], FP32, tag="o", name=f"o{t}")
                nc.vector.tensor_mul(o_t[:], pro_ps[t][:], sbc[:])
                nc.sync.dma_start(
                    out_d[t * 128:(t + 1) * 128, :], o_t[:])

            # --- main GEMM: remaining token tiles, one PSUM chain each ---
            for t in range(NPRO, NB):
                xb = xbp.tile([128, NI * 128], FP16, tag="xb", name=f"xb{t}")
                nc.sync.dma_start(xb[:], xb_d[t])
                ps_m = mp.tile([128, OC], FP32, tag="mp", name=f"pm{t}")
                for ib in range(NI):
                    nc.tensor.matmul(
                        ps_m[:], xb[:, ib * 128:(ib + 1) * 128],
                        wads[ib][:], start=(ib == 0), stop=(ib == NI - 1))
                o_t = outp.tile([128, OC], FP32, tag="o", name=f"o{t}")
                nc.vector.tensor_mul(o_t[:], ps_m[:], sbc[:])
                nc.sync.dma_start(
                    out_d[t * 128:(t + 1) * 128, :], o_t[:])

    nc.compile()
    return nc


_PROGRAM = None


def _get_program():
    global _PROGRAM
    if _PROGRAM is None:
        _PROGRAM = _build_program()
    return _PROGRAM


def _prep_inputs(x, weight, lora_a_w, lora_b_w, magnitude):
    xr = np.asarray(x, dtype=np.float32).reshape(TOK, DIN)
    wr = np.asarray(weight, dtype=np.float32)
    ar = np.asarray(lora_a_w, dtype=np.float32)
    b2 = SCALING * np.asarray(lora_b_w, dtype=np.float32)

    # x token-block-major: [NB, 128 part(i%128), NI*128] per token block
    xT = xr.T.astype(np.float16)                       # [in, tok]
    xb = np.ascontiguousarray(
        xT.reshape(NI, 128, NB, 128).transpose(2, 1, 0, 3)
        .reshape(NB, 128, NI * 128))

    wT = wr.T.astype(np.float16)                       # [in, out]
    atr = np.ascontiguousarray(ar.astype(np.float16).reshape(RANK, NI, 128))
    b2t = b2.T.astype(np.float16)                      # [rank, out]
    mag32 = magnitude.astype(np.float32).reshape(1, DOUT)

    in_maps = []
    for cpu in range(NCORES):
        cs = slice(cpu * OC, (cpu + 1) * OC)
        wt = np.ascontiguousarray(wT[:, cs].reshape(NI, 128, OC))
        in_maps.append({
            "xb": xb, "wt": wt, "atr": atr,
            "b2n": np.ascontiguousarray(b2t[:, cs]),
            "magn": np.ascontiguousarray(mag32[:, cs]),
        })
    return in_maps


def kernel(x, weight, lora_a_w, lora_b_w, magnitude, _trace=False, **_kw):
    nc = _get_program()
    in_maps = _prep_inputs(x, weight, lora_a_w, lora_b_w, magnitude)
    res = run_bass_kernel_spmd(nc, in_maps, list(range(NCORES)), trace=_trace)
    out = np.concatenate([res.results[c]["out"] for c in range(NCORES)],
                         axis=1)
    if _trace:
        kernel._last_results = res
    return out.reshape(4, 2048, DOUT)
